# revision 11
# baseline (speedup 1.0000x reference)
"""Farthest-point-sampling (npoint=2) Trainium2 Bass kernel.

Input:  xyz [1, 32, 3, 1000000] float32 (full, unsharded)
Output: centroids [32, 2] int32

Reference semantics (per batch row b):
  i0 = argmax_n y[n]                    (seed = argmax of y coordinate)
  d[n] = (x[n]-x[i0])^2 + (y[n]-y[i0])^2 + (z[n]-z[i0])^2
  i1 = argmax_n d[n]
  out[b] = [i0, i1]

Sharding: data-parallel over B — 4 rows per core on 8 cores, no
cross-core communication.
"""

import os
import sys

import numpy as np

N = 1_000_000
P = 128
F = 7813            # ceil(N / P)
NPAD = P * F        # 1000064
PADF = NPAD - N     # 64 pad elements, placed at the FRONT of partition 0
NCORES = 8
ROWS_PER_CORE = 4
B = NCORES * ROWS_PER_CORE
# y-channel pad: must never win the pass-1 argmax, and its square must stay
# finite in fp32 (the distance pad is memset before the pass-2 argmax).
YPAD = -1.0e15

_REPO_CANDIDATES = ("/opt/trn_rl_repo", "/root/.axon_site/_ro/trn_rl_repo")


def _ensure_repo_on_path():
    import importlib.util

    if importlib.util.find_spec("concourse") is not None:
        return
    for cand in _REPO_CANDIDATES:
        if os.path.isdir(cand):
            sys.path.insert(0, cand)
            return


def build_nc(rows=ROWS_PER_CORE):
    """Build the per-core Bass module (same program on all 8 cores)."""
    _ensure_repo_on_path()
    import concourse.bass as bass
    import concourse.tile as tile
    from concourse import bacc, mybir

    f32 = mybir.dt.float32
    u32 = mybir.dt.uint32
    i32 = mybir.dt.int32
    AF = mybir.ActivationFunctionType

    nc = bacc.Bacc("TRN2")
    xyzp = nc.dram_tensor("xyzp", [rows, 3, NPAD], f32, kind="ExternalInput")
    cent = nc.dram_tensor("cent", [rows, 2], i32, kind="ExternalOutput")

    with tile.TileContext(nc) as tc:
        with (
            tc.tile_pool(name="big", bufs=2) as big,
            tc.tile_pool(name="small", bufs=2) as small,
        ):
            for r in range(rows):
                X = big.tile([P, F], f32, tag="X")
                Y = big.tile([P, F], f32, tag="Y")
                Z = big.tile([P, F], f32, tag="Z")
                nc.sync.dma_start(Y[:], xyzp[r, 1].rearrange("(p f) -> p f", p=P))
                nc.sync.dma_start(X[:], xyzp[r, 0].rearrange("(p f) -> p f", p=P))
                nc.sync.dma_start(Z[:], xyzp[r, 2].rearrange("(p f) -> p f", p=P))

                outsb = small.tile([1, 2], i32, tag="outsb")

                # ---- pass 1: argmax over y ----
                m8 = small.tile([P, 8], f32, tag="m8")
                i8 = small.tile([P, 8], u32, tag="i8")
                nc.vector.max(m8[:], Y[:])
                nc.vector.max_index(i8[:], m8[:], Y[:])

                vflat = small.tile([1, P], f32, tag="vflat")
                fflat = small.tile([1, P], u32, tag="fflat")
                nc.sync.dma_start(vflat[:], m8[:, 0:1])
                nc.sync.dma_start(fflat[:], i8[:, 0:1])

                mm8 = small.tile([1, 8], f32, tag="mm8")
                pp8 = small.tile([1, 8], u32, tag="pp8")
                nc.vector.max(mm8[:], vflat[:])
                nc.vector.max_index(pp8[:], mm8[:], vflat[:])

                # NOTE: runtime asserts (value_load min/max, dynamic-AP bounds
                # checks) crash under this runtime — constrain static ranges
                # only, with skip_runtime_assert.
                pv = nc.gpsimd.value_load(pp8[0:1, 0:1])
                pv = nc.s_assert_within(pv, 0, P - 1, skip_runtime_assert=True)
                fv = nc.gpsimd.value_load(fflat[0:1, bass.ds(pv, 1)])
                fv = nc.s_assert_within(fv, 0, F - 1, skip_runtime_assert=True)
                ivp = pv * F + fv                # index into the padded layout
                iv = ivp - PADF                  # original index
                nc.gpsimd.reg_save(outsb[0:1, 0:1], iv)

                # ---- centroid gather + partition broadcast ----
                cb = small.tile([P, 3], f32, tag="cb")
                src = xyzp[r][:, bass.ds(ivp, 1)]         # [3, 1] at dynamic col
                srcT = src.transpose([1, 0])               # [1, 3]
                nc.gpsimd.dma_start(cb[:], srcT.to_broadcast((P, 3)))

                # ---- pass 2: distance and its argmax ----
                # Square(scale*v + bias) with scale=-1, bias=c gives (c-v)^2
                nc.scalar.activation(X[:], X[:], AF.Square, bias=cb[:, 0:1], scale=-1.0)
                nc.scalar.activation(Y[:], Y[:], AF.Square, bias=cb[:, 1:2], scale=-1.0)
                nc.scalar.activation(Z[:], Z[:], AF.Square, bias=cb[:, 2:3], scale=-1.0)
                nc.vector.tensor_add(X[:], X[:], Y[:])
                nc.vector.tensor_add(X[:], X[:], Z[:])
                # padded front must never win the argmax (distances are >= 0)
                nc.vector.memset(X[0:1, 0:PADF], -1.0)

                m8b = small.tile([P, 8], f32, tag="m8b")
                i8b = small.tile([P, 8], u32, tag="i8b")
                nc.vector.max(m8b[:], X[:])
                nc.vector.max_index(i8b[:], m8b[:], X[:])

                vflat2 = small.tile([1, P], f32, tag="vflat2")
                fflat2 = small.tile([1, P], u32, tag="fflat2")
                nc.sync.dma_start(vflat2[:], m8b[:, 0:1])
                nc.sync.dma_start(fflat2[:], i8b[:, 0:1])

                mm8b = small.tile([1, 8], f32, tag="mm8b")
                pp8b = small.tile([1, 8], u32, tag="pp8b")
                nc.vector.max(mm8b[:], vflat2[:])
                nc.vector.max_index(pp8b[:], mm8b[:], vflat2[:])

                pv2 = nc.gpsimd.value_load(pp8b[0:1, 0:1])
                pv2 = nc.s_assert_within(pv2, 0, P - 1, skip_runtime_assert=True)
                fv2 = nc.gpsimd.value_load(fflat2[0:1, bass.ds(pv2, 1)])
                fv2 = nc.s_assert_within(fv2, 0, F - 1, skip_runtime_assert=True)
                iv2 = pv2 * F + fv2 - PADF
                nc.gpsimd.reg_save(outsb[0:1, 1:2], iv2)

                nc.sync.dma_start(cent[r : r + 1, :], outsb[:])

    nc.compile()
    return nc


def _prep_core_input(chans):
    """chans: [rows, 3, N] float32 -> front-padded [rows, 3, NPAD]."""
    rows = chans.shape[0]
    out = np.empty((rows, 3, NPAD), dtype=np.float32)
    out[:, :, PADF:] = chans
    out[:, 0, :PADF] = 0.0
    out[:, 1, :PADF] = YPAD
    out[:, 2, :PADF] = 0.0
    return out


def kernel(xyz: np.ndarray) -> np.ndarray:
    """Full-input entry point: xyz [1, 32, 3, N] f32 -> [32, 2] int32."""
    _ensure_repo_on_path()
    from concourse import bass_utils

    xyz = np.asarray(xyz)
    assert xyz.shape == (1, B, 3, N), xyz.shape
    nc = build_nc(ROWS_PER_CORE)
    in_maps = []
    for k in range(NCORES):
        chans = np.ascontiguousarray(
            xyz[0, k * ROWS_PER_CORE : (k + 1) * ROWS_PER_CORE], dtype=np.float32
        )
        in_maps.append({"xyzp": _prep_core_input(chans)})
    res = bass_utils.run_bass_kernel_spmd(nc, in_maps, core_ids=list(range(NCORES)))
    outs = [res.results[k]["cent"].reshape(ROWS_PER_CORE, 2) for k in range(NCORES)]
    return np.concatenate(outs, axis=0).astype(np.int32)
